# revision 17
# baseline (speedup 1.0000x reference)
"""Trainium2 Bass kernel v3 for the 2-layer GAT (PyG GATConv semantics).

Strategy (8 NeuronCores, SPMD):
  - dst-node partitioning: core c owns nodes [c*SH, (c+1)*SH). Per-core table
    shard has SH+1 rows (last = pad row, a_src = -1e30).
  - dense phase per layer: node matmul with attention projections folded in
    ([h | a_src | a_dst] columns), h bf16 + a_src f32 (bitcast) in 512B rows;
    AllGather -> full table [8*(SH+1), 256] bf16.
  - edge phase: edges of each dst-batch (128 nodes) are a flat column list
    (partition = edge slot), grouped GB dst-batches per gather group; within
    a group columns are ordered (window, batch). One dma_gather call covers
    up to 8 columns of one (group, window) section; ALL slots are gathered
    (pads fetch the shard pad row) so counts are static.
  - PT one-hot ([dst->edge], ad broadcast lhsT) built on the Activation
    engine: PE broadcasts drow rows into PSUM via a ones outer product, then
    ACT does t=Abs(drow-d), PT=Relu(1-t). P01 ([edge->dst], aggregation
    lhsT) built on DVE via is_equal against an iota row.
  - weights w = exp(leaky_relu(logit)) = max(exp(l), exp(0.2 l)) with l
    clamped to +-80 (pad rows give w ~= 0); w kept bf16.
  - aggregation + softmax denominator: PE matmuls lhsT = P01 column slices,
    rhs = [w*h | w] -> PSUM accumulates [num | den] per dst over all the
    batch's columns; out = num * (1/(den+1e-30)).
"""

import numpy as np
from contextlib import ExitStack


# ---------------------------------------------------------------- config

class Cfg:
    def __init__(self, N, E, SH, GB=3):
        self.N = N
        self.E = E
        self.SH = SH                 # nodes per core (NB*128)
        self.NCORES = 8
        self.PB = 128
        self.NB = SH // 128
        self.GB = GB                 # dst batches per gather group
        self.NG = -(-self.NB // GB)
        self.SHP = SH + 1            # +1 pad row per shard
        self.NPAD = 8 * self.SHP     # table rows
        self.NW = 4
        self.WROWS = 2 * self.SHP    # rows per gather window
        self.F = 128
        self.H1, self.C1 = 4, 32
        assert SH % 128 == 0
        assert 7 * SH <= N <= 8 * SH
        assert self.WROWS <= 32768   # int16 gather index reach
        self.PADIDX = SH             # local idx of even shard's pad row


FULL = Cfg(N=100000, E=1600000, SH=12544)
MAXCALL = 8    # max 128-idx columns per dma_gather call (1024 descriptors)


# ---------------------------------------------------------------- host prep

class Sched:
    """Static cross-core-uniform edge schedule.

    Column order: group-major, then window, then batch within group.
    n_bw[b, w] = columns of section (b, w).
    """
    def __init__(self, cfg, n_bw):
        NB, NW, GB, NG = cfg.NB, cfg.NW, cfg.GB, cfg.NG
        self.n_bw = n_bw
        self.S_b = n_bw.sum(axis=1)          # cols per batch
        self.maxS = int(self.S_b.max())
        # column offsets in (g, w, b) order
        colo = np.zeros((NB, NW), np.int64)
        off = 0
        self.gcol0 = np.zeros(NG + 1, np.int64)
        self.wcol0 = np.zeros((NG, NW), np.int64)
        self.wcols = np.zeros((NG, NW), np.int64)
        for g in range(NG):
            self.gcol0[g] = off
            bs = range(g * GB, min((g + 1) * GB, NB))
            for w in range(NW):
                self.wcol0[g, w] = off
                for b in bs:
                    colo[b, w] = off
                    off += n_bw[b, w]
                self.wcols[g, w] = off - self.wcol0[g, w]
        self.gcol0[NG] = off
        self.colo = colo
        self.totcols = int(off)
        self.maxgw = int(self.wcols.max())
        self.maxgc = int((self.gcol0[1:] - self.gcol0[:-1]).max())
        # gather calls: per group, list of (w, lc0(group-local), k, c0(global))
        self.calls = []
        for g in range(NG):
            cl = []
            for w in range(NW):
                c0 = int(self.wcol0[g, w])
                left = int(self.wcols[g, w])
                while left > 0:
                    k = min(left, MAXCALL)
                    cl.append((w, c0 - int(self.gcol0[g]), k, c0))
                    c0 += k
                    left -= k
            self.calls.append(cl)
        self.maxcalls = max(len(cl) for cl in self.calls)
        # per-batch: list over w of (w, c0(global), ncols, drow_off)
        # drow_off = offset into the batch's drow_sb row (w-major concat)
        self.bsec = []
        for b in range(NB):
            secs = []
            doff = 0
            for w in range(NW):
                n = int(n_bw[b, w])
                if n > 0:
                    secs.append((w, int(colo[b, w]), n, doff))
                    doff += n * 128
            self.bsec.append(secs)


def host_prep(cfg, edge_index):
    import ml_dtypes
    N, SH, SHP, NB, NW, NC = cfg.N, cfg.SH, cfg.SHP, cfg.NB, cfg.NW, cfg.NCORES
    GB, WROWS, PADIDX = cfg.GB, cfg.WROWS, cfg.PADIDX
    src = np.concatenate([edge_index[0], np.arange(N, dtype=np.int64)])
    dst = np.concatenate([edge_index[1], np.arange(N, dtype=np.int64)])
    core_of = dst // SH

    percore = []
    counts = np.zeros((NC, NB, NW), np.int64)
    for c in range(NC):
        m = core_of == c
        ld = dst[m] - c * SH
        s = src[m]
        srow = (s // SH) * SHP + (s % SH)
        w = srow // WROWS
        b = ld >> 7
        g = b // GB
        drow = ld & 127
        order = np.lexsort((drow, b, w, g))
        b, w, drow, srow = b[order], w[order], drow[order], srow[order]
        np.add.at(counts[c], (b, w), 1)
        percore.append((b, w, drow, srow - w * WROWS))

    n_bw = -(-counts.max(axis=0) // 128)     # [NB, NW] cols
    sched = Sched(cfg, n_bw)
    tc = sched.totcols

    idx_all = np.empty((NC, 128, 8 * tc), np.int16)
    dstcol_all = np.empty((NC, 128, tc), ml_dtypes.bfloat16)
    pt_all = np.empty((NC, 128, tc * 128), ml_dtypes.bfloat16)
    for c in range(NC):
        b, w, drow, sloc = percore[c]
        # flat per-position vectors; pads (trailing within each (b, w)
        # section) gather the pad row and are excluded via dstcol = 200.
        idx_flat = np.full(tc * 128, PADIDX, np.int64)
        drow_flat = np.full(tc * 128, 200, np.int64)
        # section key monotone under the (g, w, b) sort order
        sec = ((b // GB) * NW + w) * NB + b
        first = np.searchsorted(sec, sec)
        pos = sched.colo[b, w] * 128 + (np.arange(len(sec)) - first)
        idx_flat[pos] = sloc
        drow_flat[pos] = drow
        wrp = idx_flat.astype(np.int16).reshape(8 * tc, 16).T   # [16, 8*tc]
        idx_all[c] = np.tile(wrp, (8, 1))
        dstcol_all[c] = drow_flat.reshape(tc, 128).T.astype(ml_dtypes.bfloat16)
        # PT one-hot [d, (col, e)] = (drow[col*128+e] == d); pads (200) -> 0
        pt_all[c] = (drow_flat[None, :] ==
                     np.arange(128)[:, None]).astype(ml_dtypes.bfloat16)
    return sched, idx_all, dstcol_all, pt_all


def make_wcats(cfg, W1, a_src1, a_dst1, W2, a_src2, a_dst2):
    F, H1, C1 = cfg.F, cfg.H1, cfg.C1
    W1T = np.ascontiguousarray(W1.T, dtype=np.float32)
    Bs1 = np.einsum("hck,hc->kh", W1.reshape(H1, C1, F), a_src1)
    Bd1 = np.einsum("hck,hc->kh", W1.reshape(H1, C1, F), a_dst1)
    wcat1 = np.concatenate([W1T, Bs1, Bd1], 1).astype(np.float32)   # [128,136]
    W2T = np.ascontiguousarray(W2.T, dtype=np.float32)
    Bs2 = (W2.T @ a_src2[0])[:, None]
    Bd2 = (W2.T @ a_dst2[0])[:, None]
    wcat2 = np.concatenate([W2T, Bs2, Bd2], 1).astype(np.float32)   # [128,130]
    return wcat1, wcat2


def make_core_inputs(cfg, x, sched, idx_all, dstcol_all, pt_all, wcat1,
                     wcat2, b1):
    import ml_dtypes
    N, SH, NC = cfg.N, cfg.SH, cfg.NCORES
    b1_bcast = np.broadcast_to(b1.astype(np.float32), (128, 128)).copy()
    iotas = np.zeros((128, 130), np.float32)
    iotas[:, :128] = np.arange(128, dtype=np.float32)[None, :]
    iotas[:, 128] = np.arange(128, dtype=np.float32)
    iotas[:, 129] = -np.arange(128, dtype=np.float32)
    pr = np.zeros(256, np.uint16)
    pr[128:136] = np.full(4, -1e30, np.float32).view(np.uint16)
    padrow = pr.view(ml_dtypes.bfloat16)[None, :].copy()
    maps = []
    for c in range(NC):
        base = c * SH
        cnt = min(SH, N - base)
        xs = np.zeros((SH, cfg.F), np.float32)
        xs[:cnt] = x[base:base + cnt]
        maps.append({
            "x_shard": xs,
            "idxs": idx_all[c],
            "dstcol": dstcol_all[c],
            "ptab": pt_all[c],
            "wcat1": wcat1, "wcat2": wcat2,
            "bias1": b1_bcast, "iotas": iotas, "padrow": padrow,
            "kvals": (np.arange(1, 9, dtype=np.int32) * 128)[None, :],
        })
    return maps


# ---------------------------------------------------------------- bass program

def split_multi_waits(nc):
    """This walrus build only accepts ONE embedded semaphore wait per
    instruction; move extras onto same-engine NoOps."""
    import concourse.mybir as mybir
    import bass_rust
    n_split = 0
    for f in nc.m.functions:
        for bb in f.blocks:
            lst = bb.instructions
            i = 0
            while i < len(lst):
                inst = lst[i]
                si = inst.sync_info
                if si is not None and len(si.on_wait) > 1:
                    waits = list(si.on_wait)
                    for k, w in enumerate(waits[:-1]):
                        nop = mybir.InstNoOp(name=f"{inst.name}-w{k}", ins=[], outs=[])
                        nop.engine = inst.engine
                        nop.sync_info = bass_rust.SyncInfo(on_wait=[w], on_update=[])
                        lst.insert(i, nop)
                        i += 1
                    inst.sync_info = bass_rust.SyncInfo(
                        on_wait=[waits[-1]], on_update=list(si.on_update))
                    n_split += 1
                i += 1
    return n_split


def build_bass(cfg, sched, reps=1, stages=4, split=True):
    import concourse.bass as bass
    import concourse.mybir as mybir
    import concourse.bass_isa as bass_isa
    import concourse.tile as tile
    from concourse.masks import make_identity

    fp = mybir.dt.float32
    bf = mybir.dt.bfloat16
    SH, SHP, NB, NPAD, NW = cfg.SH, cfg.SHP, cfg.NB, cfg.NPAD, cfg.NW
    GB, NG = cfg.GB, cfg.NG
    H1 = cfg.H1
    TC = sched.totcols
    AG_GROUPS = [list(range(cfg.NCORES))]
    Abs = mybir.ActivationFunctionType.Abs
    Relu = mybir.ActivationFunctionType.Relu
    Exp = mybir.ActivationFunctionType.Exp

    nc = bass.Bass(num_swdge_queues=4)
    x_shard = nc.declare_dram_parameter("x_shard", [SH, 128], fp, isOutput=False)
    idxs_d = nc.declare_dram_parameter("idxs", [128, 8 * TC], mybir.dt.int16, isOutput=False)
    dstcol_d = nc.declare_dram_parameter("dstcol", [128, TC], bf, isOutput=False)
    ptab_d = nc.declare_dram_parameter("ptab", [128, TC * 128], bf, isOutput=False)
    wcat1_d = nc.declare_dram_parameter("wcat1", [128, 136], fp, isOutput=False)
    wcat2_d = nc.declare_dram_parameter("wcat2", [128, 130], fp, isOutput=False)
    bias1_d = nc.declare_dram_parameter("bias1", [128, 128], fp, isOutput=False)
    iotas_d = nc.declare_dram_parameter("iotas", [128, 130], fp, isOutput=False)
    padrow_d = nc.declare_dram_parameter("padrow", [1, 256], bf, isOutput=False)
    kvals_d = nc.declare_dram_parameter("kvals", [1, MAXCALL], mybir.dt.int32, isOutput=False)
    out_d = nc.declare_dram_parameter("out", [SH, 128], fp, isOutput=True)

    x2_loc = nc.dram_tensor("x2_loc", [SH, 128], fp)
    ad1_loc = nc.dram_tensor("ad1_loc", [SH, 4], bf)
    ad2_loc = nc.dram_tensor("ad2_loc", [SH, 1], bf)
    haug1_loc = nc.dram_tensor("haug1_loc", [SHP, 256], bf)
    haug2_loc = nc.dram_tensor("haug2_loc", [SHP, 256], bf)
    haug1_tab = nc.dram_tensor("haug1_tab", [NPAD, 256], bf, addr_space="Shared")
    haug2_tab = nc.dram_tensor("haug2_tab", [NPAD, 256], bf, addr_space="Shared")

    def vap(t, free_dims):
        a = t[tuple([slice(None)] * len(t.shape))]
        return bass.AP(tensor=a.tensor, offset=a.offset, ap=[a.ap[0]] + free_dims)

    def vsl(a, free_dims):
        """free-dim override on an existing AP (keeps offset/partition)."""
        return bass.AP(tensor=a.tensor, offset=a.offset, ap=[a.ap[0]] + free_dims)

    with tile.TileContext(nc) as tc, ExitStack() as ctx:
        nc.gpsimd.add_instruction(bass_isa.InstPseudoReloadLibraryIndex(
            name="I-libmlp", ins=[], outs=[], lib_index=3))
        ni_regs = {}

        consts = ctx.enter_context(tc.tile_pool(name="consts", bufs=1))
        ident = consts.tile([128, 128], fp)
        make_identity(nc, ident[:])
        wc1_sb = consts.tile([128, 136], fp)
        nc.sync.dma_start(out=wc1_sb[:], in_=wcat1_d[:, :])
        wc2_sb = consts.tile([128, 130], fp)
        nc.sync.dma_start(out=wc2_sb[:], in_=wcat2_d[:, :])
        b1_sb = consts.tile([128, 128], fp)
        nc.sync.dma_start(out=b1_sb[:], in_=bias1_d[:, :])
        iotas_sb = consts.tile([128, 130], fp)
        nc.sync.dma_start(out=iotas_sb[:], in_=iotas_d[:, :])
        idx_sb = consts.tile([128, 8 * TC], mybir.dt.int16)
        nc.sync.dma_start(out=idx_sb[:], in_=idxs_d[:, :])
        dstcol_sb = consts.tile([128, TC], bf)
        nc.sync.dma_start(out=dstcol_sb[:], in_=dstcol_d[:, :])
        ones_bf = consts.tile([128, 128], bf)
        nc.vector.memset(ones_bf[:], 1.0)
        kvals_sb = consts.tile([128, MAXCALL], mybir.dt.int32)
        nc.sync.dma_start(out=kvals_sb[0:1, :], in_=kvals_d[:, :])
        for k in sorted({kk for cl in sched.calls for (_, _, kk, _) in cl}):
            ni_regs[k] = nc.gpsimd.alloc_register(f"ni_{k}")
            nc.gpsimd.reg_load(ni_regs[k], kvals_sb[0:1, k - 1:k])

        mm_x = ctx.enter_context(tc.tile_pool(name="mm_x", bufs=3))
        mm_ps = ctx.enter_context(tc.tile_pool(name="mm_ps", bufs=2, space="PSUM"))
        mm_st = ctx.enter_context(tc.tile_pool(name="mm_st", bufs=3))

        def dense_phase(src_dram, wc_sb, ncols, adW, haug_dram, ad_dram, elu_in):
            for t in range(NB):
                r0 = t * 128
                x_t = mm_x.tile([128, 128], fp, tag="x_t")
                nc.sync.dma_start(out=x_t[:], in_=src_dram[r0:r0 + 128, :])
                if elu_in:
                    z = mm_x.tile([128, 128], fp, tag="z")
                    nc.vector.tensor_tensor(out=z[:], in0=x_t[:], in1=b1_sb[:],
                                            op=mybir.AluOpType.add)
                    nc.vector.tensor_scalar_max(x_t[:], z[:], 0.0)
                    nc.vector.tensor_scalar_min(z[:], z[:], 0.0)
                    nc.scalar.activation(z[:], z[:], Exp)
                    nc.vector.tensor_tensor(out=x_t[:], in0=x_t[:], in1=z[:],
                                            op=mybir.AluOpType.add)
                    nc.vector.tensor_scalar_add(x_t[:], x_t[:], -1.0)
                xt_ps = mm_ps.tile([128, 128], fp, space="PSUM", tag="xt_ps",
                                   bufs=1)
                nc.tensor.transpose(out=xt_ps[:], in_=x_t[:], identity=ident[:])
                xt_sb = mm_x.tile([128, 128], fp, tag="xt_sb")
                nc.vector.tensor_copy(out=xt_sb[:], in_=xt_ps[:])
                o_ps = mm_ps.tile([128, ncols], fp, space="PSUM", tag="o_ps",
                                  bufs=1)
                nc.tensor.matmul(out=o_ps[:], lhsT=xt_sb[:], rhs=wc_sb[:, :ncols],
                                 start=True, stop=True)
                hb = mm_st.tile([128, 128], bf, tag="hb")
                nc.vector.tensor_copy(out=hb[:], in_=o_ps[:, 0:128])
                nc.sync.dma_start(out=haug_dram[r0:r0 + 128, 0:128], in_=hb[:])
                asb = mm_st.tile([128, adW], fp, tag="asb")
                nc.vector.tensor_copy(out=asb[:], in_=o_ps[:, 128:128 + adW])
                nc.sync.dma_start(out=haug_dram[r0:r0 + 128, 128:128 + 2 * adW],
                                  in_=asb[:].bitcast(bf))
                adb = mm_st.tile([128, adW], bf, tag="adb")
                nc.vector.tensor_copy(out=adb[:], in_=o_ps[:, 128 + adW:128 + 2 * adW])
                nc.sync.dma_start(out=ad_dram[r0:r0 + 128, :], in_=adb[:])
            nc.sync.dma_start(out=haug_dram[SH:SHP, :], in_=padrow_d[:, :])

        eg_g = ctx.enter_context(tc.tile_pool(name="eg_g", bufs=2))
        eg_p = ctx.enter_context(tc.tile_pool(name="eg_p", bufs=3))
        eg_w = ctx.enter_context(tc.tile_pool(name="eg_w", bufs=3))
        eg_s = ctx.enter_context(tc.tile_pool(name="eg_s", bufs=3))
        eg_o = ctx.enter_context(tc.tile_pool(name="eg_o", bufs=3))
        ps_b = ctx.enter_context(tc.tile_pool(name="ps_b", bufs=1, space="PSUM"))
        ps_a = ctx.enter_context(tc.tile_pool(name="ps_a", bufs=2, space="PSUM"))
        ps_g = ctx.enter_context(tc.tile_pool(name="ps_g", bufs=2, space="PSUM"))

        qcounter = [0]

        def edge_phase(tab, ad_dram, H, out_dram):
            C = 128 // H
            fl = lambda t: t[:].rearrange("p a b -> p (a b)")
            for g in range(NG):
                gc0 = int(sched.gcol0[g])
                gcols = int(sched.gcol0[g + 1]) - gc0
                bs = list(range(g * GB, min((g + 1) * GB, NB)))
                G = eg_g.tile([128, sched.maxgc, 256], bf, tag="G")
                for (w, lc0, k, c0) in sched.calls[g]:
                    nc.gpsimd.dma_gather(
                        G[:, lc0:lc0 + k, :],
                        tab[w * cfg.WROWS:(w + 1) * cfg.WROWS, :],
                        idx_sb[:, 8 * c0:8 * (c0 + k)],
                        k * 128, ni_regs[k], 256,
                        queue_num=qcounter[0] % 4)
                    qcounter[0] += 1
                # per (g, w) range: P01 on DVE; logits/weights; Gw
                # per batch: PT chunks on ACT + ad_ps matmuls; agg; out
                ad_ts = {}
                bfirst = {}
                blast = {}
                aggt = {}
                for b in bs:
                    aggt[b] = ps_g.tile([128, 128 + H], fp, space="PSUM",
                                        tag=f"agg{b % GB}", bufs=1,
                                        name=f"agg{b % GB}")
                aggs = {b: aggt[b][:, :] for b in bs}
                for b in bs:
                    ad_t = eg_s.tile([128, H], bf, tag=f"ad{b % GB}")
                    nc.sync.dma_start(out=ad_t[:],
                                      in_=ad_dram[b * 128:(b + 1) * 128, :])
                    ad_ts[b] = ad_t
                    bfirst[b] = sched.bsec[b][0][0]
                    blast[b] = sched.bsec[b][-1][0]
                for w in range(NW):
                    wc0 = int(sched.wcol0[g, w])
                    wn = int(sched.wcols[g, w])
                    if wn == 0:
                        continue
                    lw0 = wc0 - gc0
                    # P01[e, j, d] = (dstcol[e, wc0+j] == iota_d)
                    P01 = eg_p.tile([128, sched.maxgw, 128], bf, tag="P01")
                    nc.vector.tensor_tensor(
                        out=P01[:, 0:wn, :],
                        in0=vsl(dstcol_sb[:, wc0:wc0 + wn], [[1, wn], [0, 128]]),
                        in1=vap(iotas_sb, [[0, wn], [1, 128]]),
                        op=mybir.AluOpType.is_equal)
                    # PT[d, j, e] = (drow[j*128+e] == d), host-built, DMA-in
                    PT = eg_p.tile([128, sched.maxgw, 128], bf, tag="PT")
                    nc.sync.dma_start(
                        out=PT[:, 0:wn, :].rearrange("p a b -> p (a b)"),
                        in_=ptab_d[:, wc0 * 128:(wc0 + wn) * 128])
                    ad_ps = ps_a.tile([128, sched.maxgw * H], fp, space="PSUM",
                                      tag="ad_ps")
                    for b in bs:
                        secs = [s for s in sched.bsec[b] if s[0] == w]
                        if not secs:
                            continue
                        (_, c0b, nb_, doff) = secs[0]
                        lb0 = c0b - wc0     # cols of batch b within (g, w)
                        for j in range(nb_):
                            nc.tensor.matmul(
                                out=ad_ps[:, (lb0 + j) * H:(lb0 + j + 1) * H],
                                lhsT=PT[:, lb0 + j, :], rhs=ad_ts[b][:, :],
                                start=True, stop=True)
                    # logit = as (bitcast from G) + ad_e; clamp
                    logit = eg_w.tile([128, sched.maxgw, H], fp, tag="logit")
                    nc.vector.tensor_tensor(
                        out=logit[:, 0:wn, :],
                        in0=G[:, lw0:lw0 + wn, 128:128 + 2 * H].bitcast(fp),
                        in1=vsl(ad_ps[:, 0:wn * H], [[H, wn], [1, H]]),
                        op=mybir.AluOpType.add)
                    nc.vector.tensor_scalar(
                        out=logit[:, 0:wn, :], in0=logit[:, 0:wn, :],
                        scalar1=-80.0, scalar2=80.0,
                        op0=mybir.AluOpType.max, op1=mybir.AluOpType.min)
                    e1 = eg_w.tile([128, sched.maxgw, H], bf, tag="e1")
                    nc.scalar.activation(
                        vsl(fl(e1)[:, 0:wn * H], [[1, wn * H]]),
                        vsl(fl(logit)[:, 0:wn * H], [[1, wn * H]]), Exp)
                    wt = eg_w.tile([128, sched.maxgw, H], bf, tag="wt")
                    nc.scalar.activation(
                        vsl(fl(wt)[:, 0:wn * H], [[1, wn * H]]),
                        vsl(fl(logit)[:, 0:wn * H], [[1, wn * H]]), Exp,
                        scale=0.2)
                    nc.vector.tensor_tensor(out=wt[:, 0:wn, :],
                                            in0=wt[:, 0:wn, :],
                                            in1=e1[:, 0:wn, :],
                                            op=mybir.AluOpType.max)
                    # Gw[:, :, 0:128] = G.h * w (head-blocked); [128:128+H] = w
                    Gw = eg_w.tile([128, sched.maxgw, 128 + H], bf, tag="Gw")
                    nc.vector.tensor_tensor(
                        out=vap(Gw, [[128 + H, wn], [C, H], [1, C]]),
                        in0=vsl(G[:, lw0:lw0 + wn, 0:128],
                                [[256, wn], [C, H], [1, C]]),
                        in1=vap(wt, [[H, wn], [1, H], [0, C]]),
                        op=mybir.AluOpType.mult)
                    nc.vector.tensor_copy(
                        out=vsl(Gw[:, 0:wn, 128:128 + H], [[128 + H, wn], [1, H]]),
                        in_=vap(wt, [[H, wn], [1, H]]))
                    # aggregate: agg[b] += P01[:,j,:].T @ Gw[:,j,:] (PSUM
                    # accumulation spans the group's windows per batch)
                    for b in bs:
                        secs = [s for s in sched.bsec[b] if s[0] == w]
                        if not secs:
                            continue
                        (_, c0b, nb_, doff) = secs[0]
                        lb0 = c0b - wc0
                        for j in range(nb_):
                            nc.tensor.matmul(
                                out=aggs[b], lhsT=P01[:, lb0 + j, :],
                                rhs=Gw[:, lb0 + j, :],
                                start=(w == bfirst[b] and j == 0),
                                stop=(w == blast[b] and j == nb_ - 1),
                                skip_group_check=True)
                for b in bs:
                    agg_b = aggt[b]
                    den = eg_s.tile([128, H], fp, tag="den")
                    nc.vector.tensor_scalar_add(
                        den[:], agg_b[:, 128:128 + H], 1e-30)
                    rec = eg_s.tile([128, H], fp, tag="rec")
                    nc.vector.reciprocal(rec[:, :], den[:, :])
                    outt = eg_o.tile([128, 128], fp, tag="outt")
                    if H == 1:
                        nc.vector.tensor_scalar_mul(
                            outt[:, :], agg_b[:, 0:128], rec[:, 0:1])
                    else:
                        nc.vector.tensor_tensor(
                            out=vap(outt, [[C, H], [1, C]]),
                            in0=vsl(agg_b[:, 0:128], [[C, H], [1, C]]),
                            in1=vap(rec, [[1, H], [0, C]]),
                            op=mybir.AluOpType.mult)
                    nc.sync.dma_start(out=out_dram[b * 128:(b + 1) * 128, :],
                                      in_=outt[:, :])

        for _rep in range(reps):
            dense_phase(x_shard, wc1_sb, 136, 4, haug1_loc, ad1_loc, elu_in=False)
            tc.strict_bb_all_engine_barrier()
            if stages < 1:
                continue
            nc.gpsimd.collective_compute(
                "AllGather", mybir.AluOpType.bypass,
                ins=[haug1_loc[:, :]], outs=[haug1_tab[:, :]],
                replica_groups=AG_GROUPS)
            tc.strict_bb_all_engine_barrier()
            if stages >= 2:
                edge_phase(haug1_tab, ad1_loc, cfg.H1, x2_loc)
                tc.strict_bb_all_engine_barrier()
            if stages >= 3:
                dense_phase(x2_loc, wc2_sb, 130, 1, haug2_loc, ad2_loc, elu_in=True)
                tc.strict_bb_all_engine_barrier()
                nc.gpsimd.collective_compute(
                    "AllGather", mybir.AluOpType.bypass,
                    ins=[haug2_loc[:, :]], outs=[haug2_tab[:, :]],
                    replica_groups=AG_GROUPS)
                tc.strict_bb_all_engine_barrier()
            if stages >= 4:
                edge_phase(haug2_tab, ad2_loc, 1, out_d)

    import concourse.mybir as mybir2
    mybir2.codegen_inst_isa_subclasses(nc)
    if split:
        split_multi_waits(nc)
    return nc


# ---------------------------------------------------------------- entry point

def run(cfg, inputs, reps=1, stages=4, sim=False):
    x = np.asarray(inputs["x"], dtype=np.float32)
    edge_index = np.asarray(inputs["edge_index"]).astype(np.int64)
    sched, idx_all, dstcol_all, pt_all = host_prep(cfg, edge_index)
    wcat1, wcat2 = make_wcats(
        cfg, np.asarray(inputs["W1"], np.float32), np.asarray(inputs["a_src1"], np.float32),
        np.asarray(inputs["a_dst1"], np.float32), np.asarray(inputs["W2"], np.float32),
        np.asarray(inputs["a_src2"], np.float32), np.asarray(inputs["a_dst2"], np.float32))
    in_maps = make_core_inputs(cfg, x, sched, idx_all, dstcol_all, pt_all,
                               wcat1, wcat2, np.asarray(inputs["b1"], np.float32))
    nc = build_bass(cfg, sched, reps=reps, stages=stages, split=not sim)

    if sim:
        from concourse.bass_interp import MultiCoreSim
        simu = MultiCoreSim(nc, cfg.NCORES, require_finite=False,
                            require_nnan=False)
        for c in range(cfg.NCORES):
            for k, v in in_maps[c].items():
                simu.cores[c].tensor(k)[:] = v
        simu.simulate()
        results = [{"out": np.asarray(simu.cores[c].tensor("out"))}
                   for c in range(cfg.NCORES)]
    else:
        from concourse import bass2jax
        results = bass2jax.run_bass_via_pjrt(nc, in_maps, n_cores=cfg.NCORES)

    out = np.zeros((cfg.N, 128), np.float32)
    for c in range(cfg.NCORES):
        base = c * cfg.SH
        cnt = min(cfg.SH, cfg.N - base)
        out[base:base + cnt] = results[c]["out"][:cnt]
    out += np.asarray(inputs["b2"], np.float32)[None, :]
    return out


def kernel(**inputs) -> np.ndarray:
    return run(FULL, inputs)
